# revision 32
# baseline (speedup 1.0000x reference)
"""AdaAttention Trainium2 kernel: 8-way batch data parallel, fp8 DoubleRow.

Full inputs in, full outputs out. Each of the 8 NeuronCores processes a
128-row batch shard. Weights (~1.3M params) are replicated, host-packed.

v3: host-packed fp8 att in BOTH layouts (natural for the cHat MAC,
DR-transposed for the z einsum) -- no on-device casting DMAs, no XBAR
transposes, half the HBM bytes. Fused DVE ops for the score reduce.

Per-core dataflow:
  z[b,s,h] = att@W_ctx (2 fp8 DoubleRow MM, K=256 each) + h_emb via one
    identity-MM (PSUM accumulation)
  hA = tanh(z)                                (ScalarE, 2-slice batches)
  scores = affine_mul_reduce(hA, w_alpha)     (DVE, fused mul+reduce)
  shifted softmax: e = exp(scores - m_est - 2) with m_est[b] estimated on
    device from tanh(h_emb) so e fits fp8; num accumulated ON PE via
    fp8 diag(e)-pair DoubleRow matmuls into a PSUM bank; den = sum e
  out = tanh((num/den + h_lin) @ W_att2h + b) (PE + DVE + ScalarE)
"""
import numpy as np
import ml_dtypes

B = 1024
NCORES = 8
BL = B // NCORES          # 128 rows per core
S = 196                   # attention positions
D = 512                   # feature dim (RNN=ENC=HID=512)
SC = 14                   # att slices per DMA chunk
NCHUNK = S // SC          # 14
GRP = 2                   # slices per PSUM group
PAIR = 2 * GRP            # slices per exp batch
AHEAD = 3                 # chunk DMA lookahead

BF16 = ml_dtypes.bfloat16
F8 = ml_dtypes.float8_e4m3
MARGIN = 2.0              # e = exp(score - m_est - MARGIN)

_CACHE = {}


def _pack_w(w):
    # [512,512] (in,out) -> [128, 4, 512]: tile[p, dc, o] = w[dc*128+p, o]
    return np.ascontiguousarray(
        w.reshape(4, 128, D).transpose(1, 0, 2)).astype(BF16)


def _pack_w8(w):
    # [512,512] -> [128, 2, 2, 512]: tile[ki, j2, par, o] = w[2*(j2*128+ki)+par, o]
    t = w.reshape(2, 128, 2, D).transpose(1, 0, 2, 3)
    return np.clip(np.ascontiguousarray(t), -240, 240).astype(F8)


def _pack_xT(x):
    # [128, 512] -> [128, 4, 128]: tile[p, dc, b] = x[b, dc*128+p]
    return np.ascontiguousarray(
        x.reshape(128, 4, 128).transpose(2, 1, 0)).astype(BF16)


def _build(has_bz, has_bz0):
    import concourse.bass as bass
    import concourse.tile as tile
    from concourse import bacc, mybir
    from concourse.masks import make_identity

    f32 = mybir.dt.float32
    bf16 = mybir.dt.bfloat16
    fp8 = mybir.dt.float8e4
    AF = mybir.ActivationFunctionType
    DR = mybir.MatmulPerfMode.DoubleRow

    nc = bacc.Bacc("TRN2", target_bir_lowering=False, debug=False,
                   num_devices=NCORES)

    hT_ap = nc.dram_tensor("hT", [128, 4, 128], bf16, kind="ExternalInput").ap()
    sentT_ap = nc.dram_tensor("sentT", [128, 4, 128], bf16,
                              kind="ExternalInput").ap()
    att8_ap = nc.dram_tensor("att8", [BL, S, D], fp8,
                             kind="ExternalInput").ap()
    attT8_ap = nc.dram_tensor("attT8", [128, S, 2, 128, 2], fp8,
                              kind="ExternalInput").ap()
    w_aps = {}
    for name in ("wsl", "whl", "wse", "whe", "watt"):
        w_aps[name] = nc.dram_tensor(name, [128, 4, D], bf16,
                                     kind="ExternalInput").ap()
    wctx8_ap = nc.dram_tensor("wctx8", [128, 2, 2, D], fp8,
                              kind="ExternalInput").ap()
    wa4_ap = nc.dram_tensor("wa4", [128, D], bf16,
                            kind="ExternalInput").ap()
    bsl_ap = nc.dram_tensor("bsl", [128, 4], f32, kind="ExternalInput").ap()
    bhl_ap = nc.dram_tensor("bhl", [128, 4], f32, kind="ExternalInput").ap()
    brep_ap = nc.dram_tensor("brep", [128, D], f32, kind="ExternalInput").ap()
    if has_bz:
        bz_ap = nc.dram_tensor("bz", [1, D], bf16, kind="ExternalInput").ap()
    if has_bz0:
        bz0_ap = nc.dram_tensor("bz0", [1, D], bf16, kind="ExternalInput").ap()
    out_ap = nc.dram_tensor("out", [BL, D], f32, kind="ExternalOutput").ap()

    with tile.TileContext(nc) as tc:
        with tc.tile_pool(name="const", bufs=1) as constp, \
             tc.tile_pool(name="attp", bufs=5) as attp, \
             tc.tile_pool(name="attT", bufs=5) as attTp, \
             tc.tile_pool(name="hAp", bufs=4) as hAp, \
             tc.tile_pool(name="wzp", bufs=3) as wzp, \
             tc.tile_pool(name="scp", bufs=4) as scp, \
             tc.tile_pool(name="diagp", bufs=6) as diagp, \
             tc.tile_pool(name="small", bufs=1) as smallp, \
             tc.tile_pool(name="psum", bufs=3, space="PSUM") as psump, \
             tc.tile_pool(name="psumt", bufs=1, space="PSUM") as psumtp, \
             tc.tile_pool(name="nacc", bufs=1, space="PSUM") as naccp:

            # ---- all loads on the sync HWDGE queue, priority-ordered:
            # prologue weights first (small, critical path), then the z-path
            # tensors, then streamed chunks, then epilogue weights.
            att_tiles = {}
            attT_tiles = {}

            def kick_chunk(ck, eng):
                t = attTp.tile([128, SC, 2, 128, 2], fp8, tag="attT")
                eng.dma_start(out=t[:], in_=attT8_ap[:, ck * SC:(ck + 1) * SC])
                attT_tiles[ck] = t
                t2 = attp.tile([128, SC, D], fp8, tag="att8")
                eng.dma_start(out=t2[:], in_=att8_ap[:, ck * SC:(ck + 1) * SC])
                att_tiles[ck] = t2

            w = {}

            def load_w(name):
                t = constp.tile([128, 4, D], bf16, tag=name)
                nc.sync.dma_start(out=t[:], in_=w_aps[name][:])
                w[name] = t

            sentT = smallp.tile([128, 4, 128], bf16, tag="sentT")
            nc.sync.dma_start(out=sentT[:], in_=sentT_ap[:])
            load_w("wsl")
            bsl = constp.tile([128, 4], f32, tag="bsl")
            nc.sync.dma_start(out=bsl[:], in_=bsl_ap[:])
            hT = smallp.tile([128, 4, 128], bf16, tag="hT")
            nc.sync.dma_start(out=hT[:], in_=hT_ap[:])
            load_w("whl")
            bhl = constp.tile([128, 4], f32, tag="bhl")
            nc.sync.dma_start(out=bhl[:], in_=bhl_ap[:])
            load_w("whe")
            wctx8 = constp.tile([128, 2, 2, D], fp8, tag="wctx8")
            nc.sync.dma_start(out=wctx8[:], in_=wctx8_ap[:])
            load_w("wse")
            wa1 = constp.tile([128, D], bf16, tag="wa1")
            nc.sync.dma_start(out=wa1[:], in_=wa4_ap[:])
            bz = bz0 = None
            if has_bz:
                bz = constp.tile([1, D], bf16, tag="bz")
                nc.sync.dma_start(out=bz[:], in_=bz_ap[:])
            if has_bz0:
                bz0 = constp.tile([1, D], bf16, tag="bz0")
                nc.sync.dma_start(out=bz0[:], in_=bz0_ap[:])
            if has_bz or has_bz0:
                ones_row = constp.tile([1, 128], bf16, tag="ones")
                nc.vector.memset(ones_row[:], 1.0)
            kick_chunk(0, nc.sync)
            kick_chunk(1, nc.sync)
            kick_chunk(2, nc.sync)
            brep = constp.tile([128, D], f32, tag="brep")
            nc.sync.dma_start(out=brep[:], in_=brep_ap[:])
            load_w("watt")

            ident = constp.tile([128, 128], bf16, tag="ident")
            make_identity(nc, ident[:])
            ident8 = constp.tile([128, 128], fp8, tag="ident8")
            nc.vector.tensor_copy(out=ident8[:], in_=ident[:])
            e_all = constp.tile([128, 200], f32, tag="eall")

            def linear_T(xT, wt, bias_t, func, out_tag):
                """[e(P),b] = func(w[:,dc,:].T @ xT + b): transposed-layout
                linear layer. Returns bf16 [128, 4, 128] tile."""
                pt = psump.tile([128, GRP, D], f32, tag="z")
                ptf = pt.rearrange("p a b -> p (a b)")
                outT = smallp.tile([128, 4, 128], bf16, tag=out_tag)
                for ec in range(4):
                    reg = ptf[:, ec * 128:(ec + 1) * 128]
                    for dc in range(4):
                        nc.tensor.matmul(
                            reg, lhsT=wt[:, dc, ec * 128:(ec + 1) * 128],
                            rhs=xT[:, dc, :],
                            start=(dc == 0), stop=(dc == 3))
                    nc.scalar.activation(outT[:, ec, :], reg, func,
                                         bias=bias_t[:, ec:ec + 1])
                return outT

            sent_linT = linear_T(sentT, w["wsl"], bsl, AF.Relu, "slinT")
            h_linT = linear_T(hT, w["whl"], bhl, AF.Tanh, "hlinT")

            # natural-layout copies (b on partitions)
            ptn = psumtp.tile([128, GRP, D], bf16, tag="zt")
            ptn_flat = ptn.rearrange("p a b -> p (a b)")
            for ec in range(4):
                nc.tensor.transpose(ptn_flat[:, ec * 128:(ec + 1) * 128],
                                    sent_linT[:, ec, :], ident[:])
            for ec in range(4):
                nc.tensor.transpose(ptn_flat[:, D + ec * 128:D + (ec + 1) * 128],
                                    h_linT[:, ec, :], ident[:])
            sent_lin_nat8 = smallp.tile([128, D], fp8, tag="slnat8")
            nc.scalar.copy(out=sent_lin_nat8[:], in_=ptn_flat[:, 0:D])
            h_lin_nat = smallp.tile([128, D], bf16, tag="hlnat")
            nc.scalar.copy(out=h_lin_nat[:], in_=ptn_flat[:, D:2 * D])

            # h_emb precomputed once in natural layout, injected per PAIR of
            # slices with ONE identity matmul (rhs tiled x2, N=1024).
            def make_hemb(extra_bias, tag, copies):
                pt = psump.tile([128, GRP, D], f32, tag="z")
                reg = pt[:, 0, :]
                for ec in range(4):
                    nc.tensor.matmul(reg, lhsT=h_linT[:, ec, :],
                                     rhs=w["whe"][:, ec, :],
                                     start=(ec == 0),
                                     stop=(ec == 3 and extra_bias is None))
                if extra_bias is not None:
                    nc.tensor.matmul(reg, lhsT=ones_row[:],
                                     rhs=extra_bias[:], start=False, stop=True)
                out = smallp.tile([128, copies, D], bf16, tag=tag)
                for cp in range(copies):
                    nc.scalar.copy(out=out[:, cp, :], in_=reg)
                return out

            hemb_att = make_hemb(bz, "hembA", 1)
            hemb_sent = make_hemb(bz0, "hembS", 1)

            # m_est[b] = sum_h wa[h] * tanh(hemb_att[b, h]); shift = -m - 2
            th = smallp.tile([128, D], bf16, tag="mth")
            nc.scalar.activation(th[:], hemb_att[:, 0, :], AF.Tanh)
            wzm = smallp.tile([128, D], bf16, tag="mwz")
            m_est = smallp.tile([128, 1], f32, tag="mest")
            nc.vector.affine_mul_reduce(wzm[:], m_est[:], th[:], wa1[:],
                                        1.0, 0.0)
            m_shift = smallp.tile([128, 1], f32, tag="mshift")
            nc.scalar.activation(m_shift[:], m_est[:], AF.Copy,
                                 scale=-1.0, bias=-MARGIN)

            # num accumulator: one PSUM bank, PE-accumulated (fp8 DoubleRow)
            nacc = naccp.tile([128, D], f32, tag="nacc")

            # ---- sentinel slice (e-column 196) ----
            zt0 = psump.tile([128, GRP, D], f32, tag="z")
            reg0 = zt0[:, 0, :]
            for ec in range(4):
                nc.tensor.matmul(reg0, lhsT=sent_linT[:, ec, :],
                                 rhs=w["wse"][:, ec, :],
                                 start=(ec == 0), stop=False)
            nc.tensor.matmul(reg0, lhsT=ident[:], rhs=hemb_sent[:, 0, :],
                             start=False, stop=True)
            hA0 = hAp.tile([128, PAIR, D], bf16, tag="hA")
            nc.scalar.activation(hA0[:, 0, :], reg0, AF.Tanh)
            wz0 = wzp.tile([128, D], bf16, tag="wz")
            sc0 = scp.tile([128, PAIR], f32, tag="sc")
            nc.vector.affine_mul_reduce(wz0[:], sc0[:, 0:1], hA0[:, 0, :],
                                        wa1[:], 1.0, 0.0)
            nc.scalar.activation(e_all[:, 196:197], sc0[:, 0:1], AF.Exp,
                                 bias=m_shift[:])
            dg0 = diagp.tile([128, 128], fp8, tag="diag0")
            nc.vector.tensor_scalar_mul(dg0[:], ident8[:], e_all[:, 196:197])
            nc.tensor.matmul(nacc[:], lhsT=dg0[:], rhs=sent_lin_nat8[:],
                             start=True, stop=False)

            # ---- main loop over att slice pairs (MAC pipelined behind) ----
            pending = []

            def flush_pending(last=False, limit=3, max_pop=1):
                if last:
                    limit, max_pop = 0, len(pending)
                npop = 0
                while len(pending) > limit and npop < max_pop:
                    npop += 1
                    sp, dg = pending.pop(0)
                    att8 = att_tiles[sp // SC]
                    lo = sp % SC
                    nc.tensor.matmul(nacc[:], lhsT=dg[:],
                                     rhs=att8[:, lo:lo + 2, :],
                                     start=False,
                                     stop=(last and not pending),
                                     perf_mode=DR)

            NG = S // GRP                     # 98 groups of 2 slices
            GPC = SC // GRP                   # groups per chunk
            ident4 = ident8.unsqueeze(1).broadcast_to([128, PAIR, 128])
            mult = mybir.AluOpType.mult
            hA_pair = None
            for gi in range(NG):
                ck, g = divmod(gi, GPC)
                if g == 0 and ck + AHEAD < NCHUNK:
                    kick_chunk(ck + AHEAD, nc.sync)
                attT = attT_tiles[ck]
                gl = g * GRP
                sg = gi * GRP
                zt = psump.tile([128, GRP, D], f32, tag="z")
                for j in range(GRP):
                    reg = zt[:, j, :]
                    # inject first: its operands (ident, hemb) are always
                    # ready, hiding the attT LDW latency behind its stream.
                    nc.tensor.matmul(reg, lhsT=ident[:],
                                     rhs=hemb_att[:, 0, :],
                                     start=True, stop=False)
                    for par in range(2):
                        nc.tensor.matmul(
                            reg, lhsT=attT[:, gl + j, :, :, par],
                            rhs=wctx8[:, :, par, :],
                            start=False, stop=(par == 1), perf_mode=DR)
                    # spread MACs: at most one per slice, eager on last chunk
                    flush_pending(limit=1 if ck == NCHUNK - 1 else 3)
                if gi % 2 == 0:
                    hA_pair = hAp.tile([128, PAIR, D], bf16, tag="hA")
                    sc4 = scp.tile([128, PAIR], f32, tag="sc")
                    nc.scalar.activation(hA_pair[:, 0:GRP, :], zt[:], AF.Tanh)
                else:
                    nc.scalar.activation(hA_pair[:, GRP:PAIR, :], zt[:], AF.Tanh)
                for j in range(GRP):
                    jj = (gi % 2) * GRP + j
                    wz = wzp.tile([128, D], bf16, tag="wz")
                    if jj == 3:
                        # shed 1/4 of the score muls to idle GpSimd; DVE
                        # only reduces for those slices.
                        nc.gpsimd.tensor_mul(wz[:], hA_pair[:, jj, :], wa1[:])
                        nc.vector.reduce_sum(out=sc4[:, jj:jj + 1], in_=wz[:],
                                             axis=mybir.AxisListType.X)
                    else:
                        nc.vector.affine_mul_reduce(
                            wz[:], sc4[:, jj:jj + 1], hA_pair[:, jj, :],
                            wa1[:], 1.0, 0.0)
                if gi % 2 == 1:
                    sp = sg - GRP
                    nc.scalar.activation(e_all[:, sp:sp + PAIR], sc4[:], AF.Exp,
                                         bias=m_shift[:])
                    dg = diagp.tile([128, PAIR, 128], fp8, tag="diag")
                    ebc = e_all[:, sp:sp + PAIR].unsqueeze(2) \
                        .broadcast_to([128, PAIR, 128])
                    if gi >= NG - 4:
                        # tail: gpsimd's slow TT would sit on the critical
                        # path; DVE is draining by now.
                        nc.vector.tensor_mul(dg[:], ident4, ebc)
                    else:
                        nc.gpsimd.tensor_mul(dg[:], ident4, ebc)
                    pending.append((sp, dg[:, 0:GRP, :]))
                    pending.append((sp + GRP, dg[:, GRP:PAIR, :]))
            flush_pending(last=True)

            # ---- epilogue ----
            e8 = smallp.tile([128, 200], fp8, tag="e8")
            nc.vector.tensor_copy(out=e8[:, 0:197], in_=e_all[:, 0:197])
            den = smallp.tile([128, 1], f32, tag="den")
            nc.vector.reduce_sum(out=den[:], in_=e8[:, 0:197],
                                 axis=mybir.AxisListType.X)
            rec = smallp.tile([128, 1], f32, tag="rec")
            nc.vector.reciprocal(out=rec[:], in_=den[:])
            chat = smallp.tile([128, D], bf16, tag="chat")
            nc.vector.tensor_scalar_mul(chat[:], nacc[:], rec[:])
            atten_bf = smallp.tile([128, D], bf16, tag="attenbf")
            nc.vector.tensor_add(atten_bf[:], chat[:], h_lin_nat[:])
            ptf = psumtp.tile([128, GRP, D], bf16, tag="zt")
            ptf_flat = ptf.rearrange("p a b -> p (a b)")
            for dc in range(4):
                nc.tensor.transpose(ptf_flat[:, dc * 128:(dc + 1) * 128],
                                    atten_bf[:, dc * 128:(dc + 1) * 128],
                                    ident[:])
            attenT = smallp.tile([128, 4, 128], bf16, tag="attenT")
            nc.scalar.copy(out=attenT.rearrange("p a b -> p (a b)"),
                           in_=ptf_flat[:, 0:D])
            zf = psump.tile([128, GRP, D], f32, tag="z")
            regf = zf[:, 0, :]
            for dc in range(4):
                nc.tensor.matmul(regf, lhsT=attenT[:, dc, :],
                                 rhs=w["watt"][:, dc, :],
                                 start=(dc == 0), stop=(dc == 3))
            zfb = smallp.tile([128, D], f32, tag="zfb")
            nc.vector.tensor_add(zfb[:], regf, brep[:])
            out_sb = smallp.tile([128, D], f32, tag="outsb")
            nc.scalar.activation(out_sb[:], zfb[:], AF.Tanh)
            nc.scalar.dma_start(out=out_ap[:], in_=out_sb[:])

    nc.compile()
    return nc


def _prepare(h, sentinel, att_feats, W_ctx2att, b_ctx2att, W_sl, b_sl,
             W_se, b_se, W_hl, b_hl, W_he, b_he, W_alpha, b_alpha,
             W_att2h, b_att2h):
    h = np.asarray(h, dtype=np.float32)
    sentinel = np.asarray(sentinel, dtype=np.float32)
    att_feats = np.asarray(att_feats, dtype=np.float32)
    to_np = lambda a: np.asarray(a, dtype=np.float32)
    W_ctx2att, b_ctx2att = to_np(W_ctx2att), to_np(b_ctx2att)
    W_sl, b_sl = to_np(W_sl), to_np(b_sl)
    W_se, b_se = to_np(W_se), to_np(b_se)
    W_hl, b_hl = to_np(W_hl), to_np(b_hl)
    W_he, b_he = to_np(W_he), to_np(b_he)
    W_alpha, b_alpha = to_np(W_alpha), to_np(b_alpha)
    W_att2h, b_att2h = to_np(W_att2h), to_np(b_att2h)

    bias_z = b_ctx2att + b_he          # added to every att slice's z
    bias_z0 = b_se + b_he              # added to the sentinel slice's z
    has_bz = bool(np.any(bias_z))
    has_bz0 = bool(np.any(bias_z0))
    # b_alpha shifts every score equally -> cancels in softmax.

    key = (has_bz, has_bz0)
    if key not in _CACHE:
        _CACHE[key] = _build(has_bz, has_bz0)
    nc = _CACHE[key]

    wa = W_alpha[:, 0]
    shared = {
        "wsl": _pack_w(W_sl), "whl": _pack_w(W_hl), "wse": _pack_w(W_se),
        "whe": _pack_w(W_he), "watt": _pack_w(W_att2h),
        "wctx8": _pack_w8(W_ctx2att),
        "wa4": np.ascontiguousarray(
            np.broadcast_to(wa.reshape(1, D), (128, D))).astype(BF16),
        "bsl": np.ascontiguousarray(b_sl.reshape(4, 128).T).astype(np.float32),
        "bhl": np.ascontiguousarray(b_hl.reshape(4, 128).T).astype(np.float32),
        "brep": np.ascontiguousarray(
            np.broadcast_to(b_att2h.reshape(1, D), (128, D))).astype(np.float32),
    }
    if has_bz:
        shared["bz"] = bias_z.reshape(1, D).astype(BF16)
    if has_bz0:
        shared["bz0"] = bias_z0.reshape(1, D).astype(BF16)

    att8_full = np.clip(att_feats, -240, 240).astype(F8)   # [B, S, D]

    in_maps = []
    for c in range(NCORES):
        sl = slice(c * BL, (c + 1) * BL)
        m = dict(shared)
        m["hT"] = _pack_xT(h[sl])
        m["sentT"] = _pack_xT(sentinel[sl])
        a8 = att8_full[sl]                                 # [128, S, D]
        m["att8"] = np.ascontiguousarray(a8)
        # [b, s, d=(j2,k,par)] -> [k, s, j2, b, par]
        m["attT8"] = np.ascontiguousarray(
            a8.reshape(128, S, 2, 128, 2).transpose(3, 1, 2, 0, 4))
        in_maps.append(m)
    return nc, in_maps


def kernel(**inputs):
    from concourse.bass_utils import run_bass_kernel_spmd
    nc, in_maps = _prepare(**inputs)
    res = run_bass_kernel_spmd(nc, in_maps, list(range(NCORES)), trace=False)
    out = np.concatenate([res.results[i]["out"] for i in range(NCORES)], axis=0)
    return out.astype(np.float32)
